# revision 43
# baseline (speedup 1.0000x reference)
"""Multi-head causal attention (B=2, S=2048, D=1024, H=16) on 8 TRN2 cores.

Sharding: tensor-parallel over heads (4 groups of 4 heads) x data-parallel
over batch (2), one (batch, head-group) pair per core.

Per core — dual-precision fp8/bf16 pipeline:
  - Everything q-tile j>=1 touches runs fp8e4m3 with DoubleRow matmuls
    (K=256 contraction per pass): Q/K/V projections, AV over k-block pairs
    (M=66 stationary: 64 V columns + ones column for the softmax
    denominator + zero pad, since dual-fp8 LdWeights needs even M), and the
    W_O projection on fp8 attention weights/outputs scaled x16 into fp8
    normal range.
  - q-tile 0 (rows with <=512-wide attention, where fp8's relative error
    does not average out) runs a bf16 path end-to-end: bf16-projected
    Q/K over columns [0:512], bf16 V for tokens [0:512], bf16 exp weights,
    bf16 AV (M=65 with ones column), bf16 W_O. Errors: ~0.002 per partial
    in both regimes vs the 2e-2 gate.
  - scoresT[k, q] are bf16 everywhere: paired K=64 matmuls for two heads in
    the two PSUM banks of one tile (disjoint PE row groups co-execute).
  - causal masking is additive and PE-side: identity-stationary matmuls
    accumulate a -30 bias onto diagonal 128-wide sub-blocks before exp;
    no DVE masking. exp/AV/scores trim to the unmasked column range.
  - emission interleaves at k-block granularity (the attention inner loop
    is ACT/exp-paced): score steps, the previous group's AV steps, and
    filler units (projection chunks, final-proj tiles) round-robin so the
    in-order PE queue never parks behind a stalled dependency.
  - x inputs and weights are host-relaid so every DMA is 128 contiguous
    rows; x on the sync queue up front, weights on the ACT queue.
Host side: partial outputs summed over the 4 head-group cores per batch,
V-bias and output bias folded into an effective bias added at gather time.
"""

import sys

if "/opt/trn_rl_repo" not in sys.path:
    sys.path.insert(0, "/opt/trn_rl_repo")

import numpy as np
import ml_dtypes

import concourse.bass as bass
import concourse.bacc as bacc
import concourse.tile as tile
from concourse import mybir
from concourse.bass_utils import run_bass_kernel_spmd

F32 = mybir.dt.float32
BF16 = mybir.dt.bfloat16
F8E4 = mybir.dt.float8e4
DR = mybir.MatmulPerfMode.DoubleRow

P = 128
S = 2048          # sequence length
D = 1024          # model dim
C = 256           # projection columns per core (4 heads x 64)
HG = 4            # heads per core
DK = 64           # head dim
ND = 8            # d-blocks of 128 in D
NDP = 4           # d-block PAIRS
NTOK = 16         # token blocks of 128
NQ = 4            # q tiles of 512
QW = 512
MB = -30.0        # additive mask bias (exp(-30+s) ~ 0)

WSCALE = 16.0     # host premultiplier on fp8 weights
QSCALE8 = 0.125 / WSCALE
KSCALE8 = 1.0 / WSCALE
VSCALE8 = 1.0 / WSCALE
AVSCALE = 16.0    # fp8 AVT carries x16
OSCALE8 = 1.0 / (WSCALE * AVSCALE)


def build_attention_nc(causal: bool):
    nc = bacc.Bacc(None, target_bir_lowering=False)

    xq = nc.dram_tensor("xq", [P, ND * S], F8E4, kind="ExternalInput")
    xk = nc.dram_tensor("xk", [P, ND * S], F8E4, kind="ExternalInput")
    xv = nc.dram_tensor("xv", [P, ND * S], F8E4, kind="ExternalInput")
    # bf16 copies of the first 512 token columns (q-tile-0 exact path)
    xq16 = nc.dram_tensor("xq16", [P, ND * QW], BF16, kind="ExternalInput")
    xk16 = nc.dram_tensor("xk16", [P, ND * QW], BF16, kind="ExternalInput")
    xv16 = nc.dram_tensor("xv16", [P, ND * QW], BF16, kind="ExternalInput")
    wq = nc.dram_tensor("wq", [P, ND * C], F8E4, kind="ExternalInput")
    wk = nc.dram_tensor("wk", [P, ND * C], F8E4, kind="ExternalInput")
    wv = nc.dram_tensor("wv", [P, ND * C], F8E4, kind="ExternalInput")
    wo = nc.dram_tensor("wo", [P, 2 * D], F8E4, kind="ExternalInput")
    wq16 = nc.dram_tensor("wq16", [P, ND * C], BF16, kind="ExternalInput")
    wk16 = nc.dram_tensor("wk16", [P, ND * C], BF16, kind="ExternalInput")
    wv16 = nc.dram_tensor("wv16", [P, ND * C], BF16, kind="ExternalInput")
    wo16 = nc.dram_tensor("wo16", [P, 2 * D], BF16, kind="ExternalInput")
    bq = nc.dram_tensor("bq", [P, 2], F32, kind="ExternalInput")
    bk = nc.dram_tensor("bk", [P, 2], F32, kind="ExternalInput")
    ident = nc.dram_tensor("ident", [P, P], BF16, kind="ExternalInput")
    btri = nc.dram_tensor("btri", [P, P], BF16, kind="ExternalInput")
    bext = nc.dram_tensor("bext", [P, 2 * P], BF16, kind="ExternalInput")
    outT = nc.dram_tensor("outT", [D, S], BF16, kind="ExternalOutput")

    with tile.TileContext(nc) as tc:
        from contextlib import ExitStack

        with ExitStack() as ctx:
            const = ctx.enter_context(tc.tile_pool(name="const", bufs=1))
            xp = ctx.enter_context(tc.tile_pool(name="xp", bufs=24))
            resid = ctx.enter_context(tc.tile_pool(name="resid", bufs=1))
            ep8 = ctx.enter_context(tc.tile_pool(name="ep8", bufs=18))
            ep16 = ctx.enter_context(tc.tile_pool(name="ep16", bufs=10))
            dpool = ctx.enter_context(tc.tile_pool(name="dpool", bufs=2))
            opool = ctx.enter_context(tc.tile_pool(name="opool", bufs=8))
            ps_mm = ctx.enter_context(tc.tile_pool(name="ps_mm", bufs=2, space="PSUM"))
            ps_s = ctx.enter_context(tc.tile_pool(name="ps_s", bufs=2, space="PSUM"))
            ps_v = ctx.enter_context(tc.tile_pool(name="ps_v", bufs=2, space="PSUM"))

            # ---- constants on the ACT queue, in first-use order ----
            wq16_t = const.tile([P, ND, C], BF16, name="wq16_t")
            wk16_t = const.tile([P, ND, C], BF16, name="wk16_t")
            wv16_t = const.tile([P, ND, C], BF16, name="wv16_t")
            wo16_r = const.tile([P, 2, D], BF16, name="wo16_r")
            wq_t = const.tile([P, ND, C], F8E4, name="wq_t")
            wk_t = const.tile([P, ND, C], F8E4, name="wk_t")
            wv_t = const.tile([P, ND, C], F8E4, name="wv_t")
            wo_r = const.tile([P, 2, D], F8E4, name="wo_r")
            bq_t = const.tile([P, 2], F32, name="bq_t")
            bk_t = const.tile([P, 2], F32, name="bk_t")
            _xq16_t = const.tile([P, ND, QW], BF16, name="xq16_t")
            _xk16_t = const.tile([P, ND, QW], BF16, name="xk16_t")
            _xv16_t = const.tile([P, ND, QW], BF16, name="xv16_t")
            xk16_t = _xk16_t
            id_t = const.tile([P, P], BF16, name="id_t")
            btri_t = const.tile([P, P], BF16, name="btri_t")
            bext_t = const.tile([P, 2 * P], BF16, name="bext_t")
            for hh in range(2):
                nc.scalar.dma_start(
                    out=wq16_t[:, hh * 4:(hh + 1) * 4, :],
                    in_=wq16[:, hh * 4 * C:(hh + 1) * 4 * C].rearrange(
                        "p (n c) -> p n c", n=4))
            for hh in range(2):
                nc.scalar.dma_start(
                    out=wk16_t[:, hh * 4:(hh + 1) * 4, :],
                    in_=wk16[:, hh * 4 * C:(hh + 1) * 4 * C].rearrange(
                        "p (n c) -> p n c", n=4))
            nc.scalar.dma_start(out=bq_t, in_=bq[:, :])
            nc.scalar.dma_start(out=bk_t, in_=bk[:, :])
            for hh in range(4):
                nc.gpsimd.dma_start(
                    out=xk16_t[:, hh * 2:(hh + 1) * 2, :],
                    in_=xk16[:, hh * 2 * QW:(hh + 1) * 2 * QW].rearrange(
                        "p (n s) -> p n s", n=2))
            nc.gpsimd.dma_start(out=wq_t, in_=wq.rearrange("p (n c) -> p n c", n=ND))
            nc.gpsimd.dma_start(out=wk_t, in_=wk.rearrange("p (n c) -> p n c", n=ND))
            nc.gpsimd.dma_start(out=id_t, in_=ident[:, :])
            if causal:
                nc.gpsimd.dma_start(out=btri_t, in_=btri[:, :])
            nc.gpsimd.dma_start(out=wv16_t, in_=wv16.rearrange("p (n c) -> p n c", n=ND))
            nc.gpsimd.dma_start(out=wv_t, in_=wv.rearrange("p (n c) -> p n c", n=ND))
            nc.gpsimd.dma_start(out=wo_r, in_=wo.rearrange("p (n d) -> p n d", n=2))
            nc.gpsimd.dma_start(out=wo16_r, in_=wo16.rearrange("p (n d) -> p n d", n=2))

            # ---- residents ----
            QT = resid.tile([P, 2, S], BF16, name="QT")
            KT = resid.tile([P, 2, S], BF16, name="KT")
            # fp8 V per k-block pair: [pair, slot, head, 64 V | ones | pad];
            # per-head block padded to 80 so dual-fp8 LdWeights strides/
            # offsets stay 16-byte aligned
            Vp8 = resid.tile([P, NTOK // 2, 2, HG, 80], F8E4, name="Vp8")
            # bf16 V for tokens [0:512]: [tok, head, 64 V | ones]
            Vp16 = resid.tile([P, 4, HG, DK + 1], BF16, name="Vp16")
            AVT8 = resid.tile([P, 2, S], F8E4, name="AVT8")
            AVT16 = resid.tile([P, 2, QW], BF16, name="AVT16")
            nc.vector.memset(Vp8[:, :, :, :, DK:DK + 1], 1.0)
            nc.vector.memset(Vp8[:, :, :, :, DK + 1:DK + 2], 0.0)
            nc.vector.memset(Vp16[:, :, :, DK:DK + 1], 1.0)

            # ---- bf16 x tiles for the [0:512] token range ----
            xq16_t = _xq16_t
            xk16_t = _xk16_t
            xv16_t = _xv16_t
            for hh in range(4):
                nc.sync.dma_start(
                    out=xq16_t[:, hh * 2:(hh + 1) * 2, :],
                    in_=xq16[:, hh * 2 * QW:(hh + 1) * 2 * QW].rearrange(
                        "p (n s) -> p n s", n=2))
            for hh in range(4):
                nc.sync.dma_start(
                    out=xv16_t[:, hh * 2:(hh + 1) * 2, :],
                    in_=xv16[:, hh * 2 * QW:(hh + 1) * 2 * QW].rearrange(
                        "p (n s) -> p n s", n=2))

            # ---- fp8 x tiles: all issued up front on the sync queue ----
            xts = {}
            xoff = {}
            for which in ("q", "k", "v"):
                for th in range(2):
                    xts[(which, th)] = [None] * NDP
                    # q/k th0 fp8 tiles skip columns [0:512] (covered by the
                    # bf16 entry path): halves their transfer bytes
                    xoff[(which, th)] = QW if th == 0 else 0
            for th in range(2):
                for which, xdram in (("q", xq), ("k", xk), ("v", xv)):
                    off = xoff[(which, th)]
                    w = (S // 2) - off
                    for t in range(NDP):
                        xt = xp.tile([P, 2, w], F8E4,
                                     name=f"x_{which}{th}", bufs=4)
                        col = (t * 2 + th) * S
                        nc.sync.dma_start(
                            out=xt,
                            in_=xdram[:, col:col + S].rearrange(
                                "p (two s) -> p two s", two=2)[:, :, off:],
                        )
                        xts[(which, th)][t] = xt

            def proj_qk_chunk(which, th, cs, t2):
                dst = QT if which == "q" else KT
                ps = ps_mm.tile([P, QW], F32, name="mm_ps")
                if th == 0 and t2 == 0:
                    # bf16 exact path for q/k columns [0:512]
                    w_t, b_t, scale = (
                        (wq16_t, bq_t, 0.125) if which == "q" else (wk16_t, bk_t, 1.0)
                    )
                    x16 = xq16_t if which == "q" else xk16_t
                    for dd in range(ND):
                        nc.tensor.matmul(
                            ps,
                            w_t[:, dd, cs * P:(cs + 1) * P],
                            x16[:, dd, :],
                            start=(dd == 0),
                            stop=(dd == ND - 1),
                        )
                else:
                    w_t, b_t, scale = (
                        (wq_t, bq_t, QSCALE8) if which == "q" else (wk_t, bk_t, KSCALE8)
                    )
                    xt = xts[(which, th)]
                    c0 = t2 * QW - xoff[(which, th)]
                    for t in range(NDP):
                        nc.tensor.matmul(
                            ps,
                            w_t[:, 2 * t:2 * t + 2, cs * P:(cs + 1) * P],
                            xt[t][:, :, c0:c0 + QW],
                            start=(t == 0),
                            stop=(t == NDP - 1),
                            perf_mode=DR,
                        )
                nc.vector.tensor_scalar(
                    dst[:, cs, (th * 2 + t2) * QW:(th * 2 + t2 + 1) * QW],
                    ps,
                    scale,
                    b_t[:, cs:cs + 1],
                    op0=mybir.AluOpType.mult,
                    op1=mybir.AluOpType.add,
                )

            def proj_v_tile(th, t8):
                tok = th * 8 + t8
                ps = ps_mm.tile([P, QW], F32, name="mm_ps")
                if th == 0 and t8 < 4:
                    # bf16 V for tokens [0:512]; also feeds the fp8 copy
                    for dd in range(ND):
                        nc.tensor.matmul(
                            ps[:, 0:C],
                            xv16_t[:, dd, t8 * P:(t8 + 1) * P],
                            wv16_t[:, dd, :],
                            start=(dd == 0),
                            stop=(dd == ND - 1),
                        )
                    nc.vector.tensor_copy(
                        out=Vp16[:, t8, :, 0:DK],
                        in_=ps[:, 0:C].rearrange("p (h e) -> p h e", h=HG),
                    )
                    nc.vector.tensor_copy(
                        out=Vp8[:, tok // 2, tok % 2, :, 0:DK],
                        in_=ps[:, 0:C].rearrange("p (h e) -> p h e", h=HG),
                    )
                else:
                    xt = xts[("v", th)]
                    c0 = t8 * P - xoff[("v", th)]
                    for t in range(NDP):
                        nc.tensor.matmul(
                            ps[:, 0:C],
                            xt[t][:, :, c0:c0 + P],
                            wv_t[:, 2 * t:2 * t + 2, :],
                            start=(t == 0),
                            stop=(t == NDP - 1),
                            perf_mode=DR,
                        )
                    nc.vector.tensor_scalar_mul(
                        Vp8[:, tok // 2, tok % 2, :, 0:DK],
                        ps[:, 0:C].rearrange("p (h e) -> p h e", h=HG),
                        VSCALE8,
                    )

            def score_step(j, hp, kb, ets):
                dlt = kb - 4 * j
                diag = causal and dlt >= 0
                pdlt = dlt - (dlt % 2)
                # per-kb trim everywhere; the AV pair reads [pq0, 512) so the
                # odd member's [pq0, q0_own) masked span is zero-filled by a
                # cheap DVE memset instead of widened exp work
                q0 = dlt * P if (diag and dlt > 0) else 0
                if j == 0:
                    et = ep16.tile([P, 2, QW], BF16, name="e16_t")
                    ets.append((et, q0))
                elif kb % 2 == 0:
                    ep = ep8.tile([P, 2, 2, QW], F8E4, name="e8_t")
                    ets.append((ep, q0))
                elif diag and dlt % 2 == 1 and q0 > pdlt * P:
                    ep, _ = ets[kb // 2]
                    nc.vector.memset(ep[:, 1, :, pdlt * P:q0], 0.0)
                sps = ps_s.tile([P, 2, QW], F32, name="s_ps")
                for half in (0, 1):
                    rows = slice(half * DK, (half + 1) * DK)
                    nc.tensor.matmul(
                        sps[:, half, q0:QW],
                        KT[rows, hp, kb * P:(kb + 1) * P],
                        QT[rows, hp, j * QW + q0:(j + 1) * QW],
                        start=True,
                        stop=(not diag),
                    )
                if diag:
                    # PE-side additive causal mask (-30 above the diagonal)
                    for half in (0, 1):
                        nc.tensor.matmul(
                            sps[:, half, dlt * P:(dlt + 1) * P],
                            id_t, btri_t, start=False, stop=True,
                        )
                if j == 0:
                    et, _ = ets[kb]
                    nc.scalar.activation(
                        et[:, :, q0:QW], sps[:, :, q0:QW],
                        mybir.ActivationFunctionType.Exp,
                    )
                else:
                    ep, _ = ets[kb // 2]
                    nc.scalar.activation(
                        ep[:, kb % 2, :, q0:QW], sps[:, :, q0:QW],
                        mybir.ActivationFunctionType.Exp,
                    )

            def emit_readout(j, hp, half, avp, last=False):
                qs = slice(j * QW, (j + 1) * QW)
                avs = dpool.tile([DK, QW], F32, name="avs_t")
                den = dpool.tile([1, QW], F32, name="den_t")
                dscale = (1.0 / AVSCALE) if j != 0 else 1.0
                if last:
                    # ACT is idle after the final exp: evacuate PSUM there
                    nc.scalar.copy(out=avs, in_=avp[0:DK, :])
                    nc.scalar.mul(den, avp[DK:DK + 1, :], dscale)
                else:
                    nc.vector.tensor_copy(out=avs, in_=avp[0:DK, :])
                    # fold the fp8 x16 AVT scale into the denominator so the
                    # normalize is a plain tensor_mul
                    nc.vector.tensor_scalar_mul(den, avp[DK:DK + 1, :], dscale)
                rec = dpool.tile([1, 1, QW], F32, name="rec_t")
                nc.vector.reciprocal_approx_fast(out=rec[:, 0, :], in_=den)
                bc = dpool.tile([DK, QW], F32, name="bc_t")
                # partition-broadcast via a tiny SBUF->SBUF DMA (0-stride
                # source row) instead of the Pool DSP op + its drains
                nc.gpsimd.dma_start(out=bc, in_=rec.to_broadcast((1, DK, QW)))
                dst = (AVT16[half * DK:(half + 1) * DK, hp, :] if j == 0
                       else AVT8[half * DK:(half + 1) * DK, hp, qs])
                nc.vector.tensor_mul(dst, avs, bc)

            def av_generator(j, hp, ets, on_done):
                for half in (0, 1):
                    h = 2 * hp + half
                    avp = ps_v.tile([P, QW], F32, name="av_ps")
                    n = len(ets)
                    if j == 0:
                        for kb, (et, q0) in enumerate(ets):
                            nc.tensor.matmul(
                                avp[0:DK + 1, q0:QW],
                                Vp16[:, kb, h, :],
                                et[:, half, q0:QW],
                                start=(kb == 0),
                                stop=(kb == n - 1),
                            )
                            yield
                    else:
                        for i, (ep, q0) in enumerate(ets):
                            nc.tensor.matmul(
                                avp[0:DK + 2, q0:QW],
                                Vp8[:, i, :, h, 0:DK + 2],
                                ep[:, :, half, q0:QW],
                                start=(i == 0),
                                stop=(i == n - 1),
                                perf_mode=DR,
                            )
                            yield
                    emit_readout(j, hp, half, avp,
                                 last=((j, hp) == (2, 1) and causal))
                on_done()

            def final_unit(qn, m):
                ps = ps_mm.tile([P, QW], F32, name="mm_ps")
                ot = opool.tile([P, QW], BF16, name="o_t")
                if qn == 0:
                    for cs in range(2):
                        nc.tensor.matmul(
                            ps,
                            wo16_r[:, cs, m * P:(m + 1) * P],
                            AVT16[:, cs, :],
                            start=(cs == 0),
                            stop=(cs == 1),
                        )
                    nc.vector.tensor_copy(out=ot, in_=ps)
                    nc.sync.dma_start(
                        out=outT[m * P:(m + 1) * P, 0:QW], in_=ot)
                elif qn == 2:
                    # last-emitted group: drain in parallel — even tiles on
                    # the then-idle ACT engine + its queue, odd tiles on DVE
                    # + sync, halving the serial tail chain
                    nc.tensor.matmul(
                        ps,
                        wo_r[:, :, m * P:(m + 1) * P],
                        AVT8[:, :, qn * QW:(qn + 1) * QW],
                        start=True, stop=True, perf_mode=DR,
                    )
                    if m % 2 == 0:
                        nc.scalar.mul(ot, ps, OSCALE8)
                        nc.scalar.dma_start(
                            out=outT[m * P:(m + 1) * P, qn * QW:(qn + 1) * QW],
                            in_=ot)
                    else:
                        nc.vector.tensor_scalar_mul(ot, ps, OSCALE8)
                        nc.sync.dma_start(
                            out=outT[m * P:(m + 1) * P, qn * QW:(qn + 1) * QW],
                            in_=ot)
                else:
                    nc.tensor.matmul(
                        ps,
                        wo_r[:, :, m * P:(m + 1) * P],
                        AVT8[:, :, qn * QW:(qn + 1) * QW],
                        start=True, stop=True, perf_mode=DR,
                    )
                    nc.vector.tensor_scalar_mul(ot, ps, OSCALE8)
                    nc.sync.dma_start(
                        out=outT[m * P:(m + 1) * P, qn * QW:(qn + 1) * QW],
                        in_=ot)

            # ---------- schedule ----------
            filler = []
            released = set()

            def fill(n=1):
                done = 0
                i = 0
                while done < n and i < len(filler):
                    tag, fn = filler[i]
                    if tag and tag not in released:
                        i += 1
                        continue
                    filler.pop(i)
                    fn()
                    done += 1

            def mk_chunk(which, th, cs, t2):
                return lambda: proj_qk_chunk(which, th, cs, t2)

            def mk_v(th, t8):
                return lambda: proj_v_tile(th, t8)

            def mk_f(qn, m):
                return lambda: final_unit(qn, m)

            # warm up the PE during the initial DMA wait: the clock needs
            # ~3us of continuous execution to ramp 0.65 -> 2.4 GHz, so burn
            # dummy matmuls on a memset tile until the entry inputs land
            warm = const.tile([P, DK], BF16, name="warm_t")
            nc.vector.memset(warm, 0.125)
            for _ in range(64):
                wps = ps_mm.tile([P, QW], F32, name="mm_ps")
                nc.tensor.matmul(wps[0:DK, 0:DK], warm, warm,
                                 start=True, stop=True)

            # entry: only what S(0,0) reads (q/k columns [0:512], head
            # pair 0, bf16 path); everything else is filler in first-use
            # order, emitted AFTER each score step so scores lead the queue
            proj_qk_chunk("q", 0, 0, 0)
            proj_qk_chunk("k", 0, 0, 0)
            filler.append(("", mk_chunk("q", 0, 1, 0)))   # S(0,1)
            filler.append(("", mk_chunk("k", 0, 1, 0)))
            for t8 in range(8):
                filler.append(("", mk_v(0, t8)))          # A(0,*)
            filler.append(("", mk_chunk("q", 0, 0, 1)))   # S(1,0)
            filler.append(("", mk_chunk("k", 0, 0, 1)))
            filler.append(("", mk_chunk("q", 0, 1, 1)))   # S(1,1)
            filler.append(("", mk_chunk("k", 0, 1, 1)))
            for t2 in (0, 1):
                filler.append(("", mk_chunk("q", 1, 0, t2)))
            for t2 in (0, 1):
                filler.append(("", mk_chunk("k", 1, 0, t2)))
            for t2 in (0, 1):
                filler.append(("", mk_chunk("q", 1, 1, t2)))
            for t2 in (0, 1):
                filler.append(("", mk_chunk("k", 1, 1, t2)))
            for t8 in range(8):
                filler.append(("", mk_v(1, t8)))
            for qn in range(NQ):
                for m in range(ND):
                    filler.append((f"F{qn}", mk_f(qn, m)))

            groups = [(0, 0), (0, 1), (1, 0), (1, 1), (3, 0), (3, 1), (2, 0), (2, 1)]
            prev_gen = None
            prev_n = 0
            for (j, hp) in groups:
                nkb = 4 * j + 4 if causal else NTOK
                ets = []
                done_av = 0
                for kb in range(nkb):
                    score_step(j, hp, kb, ets)
                    if prev_gen is not None:
                        target = ((kb + 1) * prev_n) // nkb
                        while done_av < target:
                            next(prev_gen, None)
                            done_av += 1
                    fill(2)
                if prev_gen is not None:
                    while done_av < prev_n:
                        next(prev_gen, None)
                        done_av += 1
                    next(prev_gen, None)  # trailing readout + on_done

                def mk_done(jj):
                    return lambda: released.add(f"F{jj}")

                prev_gen = av_generator(j, hp, ets,
                                        mk_done(j) if hp == 1 else (lambda: None))
                prev_n = 2 * len(ets)
            for _ in range(prev_n):
                next(prev_gen, None)
                fill(1)
            next(prev_gen, None)
            fill(len(filler) + 1)

    nc.compile()
    return nc


_NC_CACHE = {}


def _get_nc(causal: bool):
    if causal not in _NC_CACHE:
        _NC_CACHE[causal] = build_attention_nc(causal)
    return _NC_CACHE[causal]


def _relay_w(WT):
    # WT is (d_in, c): -> [128, (d, c)] so each partition row is contiguous
    return np.ascontiguousarray(
        WT.reshape(ND, P, -1).transpose(1, 0, 2).reshape(P, -1)
    )


def _relay_x(xT):
    # xT is (D, S): -> [128, (t, th, two, s)] matching x-tile DMA slices
    a = xT.reshape(NDP, 2, P, 2, S // 2)        # t, two, p, th, s
    return np.ascontiguousarray(
        a.transpose(2, 0, 3, 1, 4).reshape(P, ND * S)
    )


def build_in_maps(query, key, value, Wq, bq, Wk, bk, Wv, Wo, causal):
    f8 = ml_dtypes.float8_e4m3fn
    bf = ml_dtypes.bfloat16
    kk = np.arange(P)[:, None]
    qq = np.arange(P)[None, :]
    tri = np.where(kk > qq, np.float32(MB), np.float32(0.0))
    if not causal:
        tri = np.zeros((P, P), np.float32)
    bext_np = np.concatenate([np.full((P, P), MB, np.float32), tri], axis=1)
    ident_np = np.eye(P, dtype=np.float32)

    xT = {n: [np.ascontiguousarray(a[b].T) for b in range(2)]
          for n, a in (("q", query), ("k", key), ("v", value))}
    x8 = {n: [_relay_x(xT[n][b]).astype(f8) for b in range(2)] for n in xT}
    x16 = {n: [_relay_w(xT[n][b][:, 0:QW]).astype(bf) for b in range(2)] for n in xT}

    WqT = np.ascontiguousarray(Wq.T)
    WkT = np.ascontiguousarray(Wk.T)
    WvT = np.ascontiguousarray(Wv.T)
    WoT = np.ascontiguousarray(Wo.T)

    in_maps = []
    for core in range(8):
        b, g = divmod(core, 4)
        cols = slice(g * C, (g + 1) * C)
        wo_core = WoT[cols, :]  # (256, 1024)

        def relay_wo(w):
            return np.ascontiguousarray(
                w.reshape(2, P, D).transpose(1, 0, 2).reshape(P, 2 * D)
            )
        in_maps.append({
            "xq": x8["q"][b], "xk": x8["k"][b], "xv": x8["v"][b],
            "xq16": x16["q"][b], "xk16": x16["k"][b], "xv16": x16["v"][b],
            "wq": _relay_w(WqT[:, cols] * WSCALE).astype(f8),
            "wk": _relay_w(WkT[:, cols] * WSCALE).astype(f8),
            "wv": _relay_w(WvT[:, cols] * WSCALE).astype(f8),
            "wo": relay_wo(wo_core * WSCALE).astype(f8),
            "wq16": _relay_w(WqT[:, cols]).astype(bf),
            "wk16": _relay_w(WkT[:, cols]).astype(bf),
            "wv16": _relay_w(WvT[:, cols]).astype(bf),
            "wo16": relay_wo(wo_core).astype(bf),
            "bq": np.ascontiguousarray((bq[cols] / 8.0).reshape(2, P).T),
            "bk": np.ascontiguousarray(bk[cols].reshape(2, P).T),
            "ident": ident_np.astype(bf),
            "btri": tri.astype(bf),
            "bext": bext_np.astype(bf),
        })
    return in_maps


def kernel(query, key, value, mask, Wq, bq, Wk, bk, Wv, bv, Wo, bo):
    query = np.asarray(query, np.float32)
    key = np.asarray(key, np.float32)
    value = np.asarray(value, np.float32)
    Wq = np.asarray(Wq, np.float32)
    Wk = np.asarray(Wk, np.float32)
    Wv = np.asarray(Wv, np.float32)
    Wo = np.asarray(Wo, np.float32)
    bq = np.asarray(bq, np.float32)
    bk = np.asarray(bk, np.float32)
    bv = np.asarray(bv, np.float32)
    bo = np.asarray(bo, np.float32)
    mask_np = np.asarray(mask)

    causal = bool(mask_np.any())
    if causal:
        idx = np.arange(S)
        expect = idx[None, :] > idx[:, None]
        if not np.array_equal(mask_np.reshape(S, S), expect):
            raise ValueError("kernel only supports the causal (or empty) mask")
    nc = _get_nc(causal)

    in_maps = build_in_maps(query, key, value, Wq, bq, Wk, bk, Wv, Wo, causal)

    res = run_bass_kernel_spmd(nc, in_maps, core_ids=list(range(8)))

    # softmax rows sum to 1, so the V bias contributes bv @ Wo.T to every row.
    bo_eff = bo + bv @ Wo.T
    out = np.empty((2, S, D), np.float32)
    for b in range(2):
        acc = res.results[b * 4]["outT"].astype(np.float32)
        for g in range(1, 4):
            acc += res.results[b * 4 + g]["outT"].astype(np.float32)
        out[b] = acc.T.astype(np.float32) + bo_eff
    return out


# revision 44
# speedup vs baseline: 1.3576x; 1.3576x over previous
"""Multi-head causal attention (B=2, S=2048, D=1024, H=16) on 8 TRN2 cores.

Sharding: tensor-parallel over heads (4 groups of 4 heads) x data-parallel
over batch (2), one (batch, head-group) pair per core.

Per core — dual-precision fp8/bf16 pipeline:
  - Everything q-tile j>=1 touches runs fp8e4m3 with DoubleRow matmuls
    (K=256 contraction per pass): Q/K/V projections, AV over k-block pairs
    (M=66 stationary: 64 V columns + ones column for the softmax
    denominator + zero pad, since dual-fp8 LdWeights needs even M), and the
    W_O projection on fp8 attention weights/outputs scaled x16 into fp8
    normal range.
  - q-tile 0 (rows with <=512-wide attention, where fp8's relative error
    does not average out) runs a bf16 path end-to-end: bf16-projected
    Q/K over columns [0:512], bf16 V for tokens [0:512], bf16 exp weights,
    bf16 AV (M=65 with ones column), bf16 W_O. Errors: ~0.002 per partial
    in both regimes vs the 2e-2 gate.
  - scoresT[k, q] are bf16 everywhere: paired K=64 matmuls for two heads in
    the two PSUM banks of one tile (disjoint PE row groups co-execute).
  - causal masking is additive and PE-side: identity-stationary matmuls
    accumulate a -30 bias onto diagonal 128-wide sub-blocks before exp;
    no DVE masking. exp/AV/scores trim to the unmasked column range.
  - emission interleaves at k-block granularity (the attention inner loop
    is ACT/exp-paced): score steps, the previous group's AV steps, and
    filler units (projection chunks, final-proj tiles) round-robin so the
    in-order PE queue never parks behind a stalled dependency.
  - x inputs and weights are host-relaid so every DMA is 128 contiguous
    rows; x on the sync queue up front, weights on the ACT queue.
Host side: partial outputs summed over the 4 head-group cores per batch,
V-bias and output bias folded into an effective bias added at gather time.
"""

import sys

if "/opt/trn_rl_repo" not in sys.path:
    sys.path.insert(0, "/opt/trn_rl_repo")

import numpy as np
import ml_dtypes

import concourse.bass as bass
import concourse.bacc as bacc
import concourse.tile as tile
from concourse import mybir
from concourse.bass_utils import run_bass_kernel_spmd

F32 = mybir.dt.float32
BF16 = mybir.dt.bfloat16
F8E4 = mybir.dt.float8e4
DR = mybir.MatmulPerfMode.DoubleRow

P = 128
S = 2048          # sequence length
D = 1024          # model dim
C = 256           # projection columns per core (4 heads x 64)
HG = 4            # heads per core
DK = 64           # head dim
ND = 8            # d-blocks of 128 in D
NDP = 4           # d-block PAIRS
NTOK = 16         # token blocks of 128
NQ = 4            # q tiles of 512
QW = 512
MB = -30.0        # additive mask bias (exp(-30+s) ~ 0)

WSCALE = 16.0     # host premultiplier on fp8 weights
QSCALE8 = 0.125 / WSCALE
KSCALE8 = 1.0 / WSCALE
VSCALE8 = 1.0 / WSCALE
AVSCALE = 16.0    # fp8 AVT carries x16
OSCALE8 = 1.0 / (WSCALE * AVSCALE)


def build_attention_nc(causal: bool):
    nc = bacc.Bacc(None, target_bir_lowering=False)

    xq = nc.dram_tensor("xq", [P, ND * S], F8E4, kind="ExternalInput")
    xk = nc.dram_tensor("xk", [P, ND * S], F8E4, kind="ExternalInput")
    xv = nc.dram_tensor("xv", [P, ND * S], F8E4, kind="ExternalInput")
    # bf16 copies of the first 512 token columns (q-tile-0 exact path)
    xq16 = nc.dram_tensor("xq16", [P, ND * QW], BF16, kind="ExternalInput")
    xk16 = nc.dram_tensor("xk16", [P, ND * QW], BF16, kind="ExternalInput")
    xv16 = nc.dram_tensor("xv16", [P, ND * QW], BF16, kind="ExternalInput")
    wq = nc.dram_tensor("wq", [P, ND * C], F8E4, kind="ExternalInput")
    wk = nc.dram_tensor("wk", [P, ND * C], F8E4, kind="ExternalInput")
    wv = nc.dram_tensor("wv", [P, ND * C], F8E4, kind="ExternalInput")
    wo = nc.dram_tensor("wo", [P, 2 * D], F8E4, kind="ExternalInput")
    wq16 = nc.dram_tensor("wq16", [P, ND * C], BF16, kind="ExternalInput")
    wk16 = nc.dram_tensor("wk16", [P, ND * C], BF16, kind="ExternalInput")
    wv16 = nc.dram_tensor("wv16", [P, ND * C], BF16, kind="ExternalInput")
    wo16 = nc.dram_tensor("wo16", [P, 2 * D], BF16, kind="ExternalInput")
    bq = nc.dram_tensor("bq", [P, 2], F32, kind="ExternalInput")
    bk = nc.dram_tensor("bk", [P, 2], F32, kind="ExternalInput")
    ident = nc.dram_tensor("ident", [P, P], BF16, kind="ExternalInput")
    btri = nc.dram_tensor("btri", [P, P], BF16, kind="ExternalInput")
    bext = nc.dram_tensor("bext", [P, 2 * P], BF16, kind="ExternalInput")
    outT = nc.dram_tensor("outT", [D, S], BF16, kind="ExternalOutput")

    with tile.TileContext(nc) as tc:
        from contextlib import ExitStack

        with ExitStack() as ctx:
            const = ctx.enter_context(tc.tile_pool(name="const", bufs=1))
            xp = ctx.enter_context(tc.tile_pool(name="xp", bufs=24))
            resid = ctx.enter_context(tc.tile_pool(name="resid", bufs=1))
            ep8 = ctx.enter_context(tc.tile_pool(name="ep8", bufs=18))
            ep16 = ctx.enter_context(tc.tile_pool(name="ep16", bufs=10))
            dpool = ctx.enter_context(tc.tile_pool(name="dpool", bufs=2))
            opool = ctx.enter_context(tc.tile_pool(name="opool", bufs=8))
            ps_mm = ctx.enter_context(tc.tile_pool(name="ps_mm", bufs=2, space="PSUM"))
            ps_s = ctx.enter_context(tc.tile_pool(name="ps_s", bufs=2, space="PSUM"))
            ps_v = ctx.enter_context(tc.tile_pool(name="ps_v", bufs=2, space="PSUM"))

            # ---- constants on the ACT queue, in first-use order ----
            wq16_t = const.tile([P, ND, C], BF16, name="wq16_t")
            wk16_t = const.tile([P, ND, C], BF16, name="wk16_t")
            wv16_t = const.tile([P, ND, C], BF16, name="wv16_t")
            wo16_r = const.tile([P, 2, D], BF16, name="wo16_r")
            wq_t = const.tile([P, ND, C], F8E4, name="wq_t")
            wk_t = const.tile([P, ND, C], F8E4, name="wk_t")
            wv_t = const.tile([P, ND, C], F8E4, name="wv_t")
            wo_r = const.tile([P, 2, D], F8E4, name="wo_r")
            bq_t = const.tile([P, 2], F32, name="bq_t")
            bk_t = const.tile([P, 2], F32, name="bk_t")
            _xq16_t = const.tile([P, ND, QW], BF16, name="xq16_t")
            _xk16_t = const.tile([P, ND, QW], BF16, name="xk16_t")
            _xv16_t = const.tile([P, ND, QW], BF16, name="xv16_t")
            xk16_t = _xk16_t
            id_t = const.tile([P, P], BF16, name="id_t")
            btri_t = const.tile([P, P], BF16, name="btri_t")
            bext_t = const.tile([P, 2 * P], BF16, name="bext_t")
            for hh in range(2):
                nc.scalar.dma_start(
                    out=wq16_t[:, hh * 4:(hh + 1) * 4, :],
                    in_=wq16[:, hh * 4 * C:(hh + 1) * 4 * C].rearrange(
                        "p (n c) -> p n c", n=4))
            for hh in range(2):
                nc.scalar.dma_start(
                    out=wk16_t[:, hh * 4:(hh + 1) * 4, :],
                    in_=wk16[:, hh * 4 * C:(hh + 1) * 4 * C].rearrange(
                        "p (n c) -> p n c", n=4))
            nc.scalar.dma_start(out=bq_t, in_=bq[:, :])
            nc.scalar.dma_start(out=bk_t, in_=bk[:, :])
            for hh in range(4):
                nc.gpsimd.dma_start(
                    out=xk16_t[:, hh * 2:(hh + 1) * 2, :],
                    in_=xk16[:, hh * 2 * QW:(hh + 1) * 2 * QW].rearrange(
                        "p (n s) -> p n s", n=2))
            nc.gpsimd.dma_start(out=wq_t, in_=wq.rearrange("p (n c) -> p n c", n=ND))
            nc.gpsimd.dma_start(out=wk_t, in_=wk.rearrange("p (n c) -> p n c", n=ND))
            nc.gpsimd.dma_start(out=id_t, in_=ident[:, :])
            if causal:
                nc.gpsimd.dma_start(out=btri_t, in_=btri[:, :])
            nc.gpsimd.dma_start(out=wv16_t, in_=wv16.rearrange("p (n c) -> p n c", n=ND))
            nc.gpsimd.dma_start(out=wv_t, in_=wv.rearrange("p (n c) -> p n c", n=ND))
            nc.gpsimd.dma_start(out=wo_r, in_=wo.rearrange("p (n d) -> p n d", n=2))
            nc.gpsimd.dma_start(out=wo16_r, in_=wo16.rearrange("p (n d) -> p n d", n=2))

            # ---- residents ----
            QT = resid.tile([P, 2, S], BF16, name="QT")
            KT = resid.tile([P, 2, S], BF16, name="KT")
            # fp8 V per k-block pair: [pair, slot, head, 64 V | ones | pad];
            # per-head block padded to 80 so dual-fp8 LdWeights strides/
            # offsets stay 16-byte aligned
            Vp8 = resid.tile([P, NTOK // 2, 2, HG, 80], F8E4, name="Vp8")
            # bf16 V for tokens [0:512]: [tok, head, 64 V | ones]
            Vp16 = resid.tile([P, 4, HG, DK + 1], BF16, name="Vp16")
            AVT8 = resid.tile([P, 2, S], F8E4, name="AVT8")
            AVT16 = resid.tile([P, 2, QW], BF16, name="AVT16")
            nc.vector.memset(Vp8[:, :, :, :, DK:DK + 1], 1.0)
            nc.vector.memset(Vp8[:, :, :, :, DK + 1:DK + 2], 0.0)
            nc.vector.memset(Vp16[:, :, :, DK:DK + 1], 1.0)

            # ---- bf16 x tiles for the [0:512] token range ----
            xq16_t = _xq16_t
            xk16_t = _xk16_t
            xv16_t = _xv16_t
            for hh in range(4):
                nc.sync.dma_start(
                    out=xq16_t[:, hh * 2:(hh + 1) * 2, :],
                    in_=xq16[:, hh * 2 * QW:(hh + 1) * 2 * QW].rearrange(
                        "p (n s) -> p n s", n=2))
            for hh in range(4):
                nc.sync.dma_start(
                    out=xv16_t[:, hh * 2:(hh + 1) * 2, :],
                    in_=xv16[:, hh * 2 * QW:(hh + 1) * 2 * QW].rearrange(
                        "p (n s) -> p n s", n=2))

            # ---- fp8 x tiles: all issued up front on the sync queue ----
            xts = {}
            xoff = {}
            for which in ("q", "k", "v"):
                for th in range(2):
                    xts[(which, th)] = [None] * NDP
                    # q/k th0 fp8 tiles skip columns [0:512] (covered by the
                    # bf16 entry path): halves their transfer bytes
                    xoff[(which, th)] = QW if th == 0 else 0
            for th in range(2):
                for which, xdram in (("q", xq), ("k", xk), ("v", xv)):
                    off = xoff[(which, th)]
                    w = (S // 2) - off
                    for t in range(NDP):
                        xt = xp.tile([P, 2, w], F8E4,
                                     name=f"x_{which}{th}", bufs=4)
                        col = (t * 2 + th) * S
                        nc.sync.dma_start(
                            out=xt,
                            in_=xdram[:, col:col + S].rearrange(
                                "p (two s) -> p two s", two=2)[:, :, off:],
                        )
                        xts[(which, th)][t] = xt

            def proj_qk_chunk(which, th, cs, t2):
                dst = QT if which == "q" else KT
                ps = ps_mm.tile([P, QW], F32, name="mm_ps")
                if th == 0 and t2 == 0:
                    # bf16 exact path for q/k columns [0:512]
                    w_t, b_t, scale = (
                        (wq16_t, bq_t, 0.125) if which == "q" else (wk16_t, bk_t, 1.0)
                    )
                    x16 = xq16_t if which == "q" else xk16_t
                    for dd in range(ND):
                        nc.tensor.matmul(
                            ps,
                            w_t[:, dd, cs * P:(cs + 1) * P],
                            x16[:, dd, :],
                            start=(dd == 0),
                            stop=(dd == ND - 1),
                        )
                else:
                    w_t, b_t, scale = (
                        (wq_t, bq_t, QSCALE8) if which == "q" else (wk_t, bk_t, KSCALE8)
                    )
                    xt = xts[(which, th)]
                    c0 = t2 * QW - xoff[(which, th)]
                    for t in range(NDP):
                        nc.tensor.matmul(
                            ps,
                            w_t[:, 2 * t:2 * t + 2, cs * P:(cs + 1) * P],
                            xt[t][:, :, c0:c0 + QW],
                            start=(t == 0),
                            stop=(t == NDP - 1),
                            perf_mode=DR,
                        )
                nc.vector.tensor_scalar(
                    dst[:, cs, (th * 2 + t2) * QW:(th * 2 + t2 + 1) * QW],
                    ps,
                    scale,
                    b_t[:, cs:cs + 1],
                    op0=mybir.AluOpType.mult,
                    op1=mybir.AluOpType.add,
                )

            def proj_v_tile(th, t8):
                tok = th * 8 + t8
                ps = ps_mm.tile([P, QW], F32, name="mm_ps")
                if th == 0 and t8 < 4:
                    # bf16 V for tokens [0:512]; also feeds the fp8 copy
                    for dd in range(ND):
                        nc.tensor.matmul(
                            ps[:, 0:C],
                            xv16_t[:, dd, t8 * P:(t8 + 1) * P],
                            wv16_t[:, dd, :],
                            start=(dd == 0),
                            stop=(dd == ND - 1),
                        )
                    nc.vector.tensor_copy(
                        out=Vp16[:, t8, :, 0:DK],
                        in_=ps[:, 0:C].rearrange("p (h e) -> p h e", h=HG),
                    )
                    nc.vector.tensor_copy(
                        out=Vp8[:, tok // 2, tok % 2, :, 0:DK],
                        in_=ps[:, 0:C].rearrange("p (h e) -> p h e", h=HG),
                    )
                else:
                    xt = xts[("v", th)]
                    c0 = t8 * P - xoff[("v", th)]
                    for t in range(NDP):
                        nc.tensor.matmul(
                            ps[:, 0:C],
                            xt[t][:, :, c0:c0 + P],
                            wv_t[:, 2 * t:2 * t + 2, :],
                            start=(t == 0),
                            stop=(t == NDP - 1),
                            perf_mode=DR,
                        )
                    nc.vector.tensor_scalar_mul(
                        Vp8[:, tok // 2, tok % 2, :, 0:DK],
                        ps[:, 0:C].rearrange("p (h e) -> p h e", h=HG),
                        VSCALE8,
                    )

            def score_step(j, hp, kb, ets):
                dlt = kb - 4 * j
                diag = causal and dlt >= 0
                pdlt = dlt - (dlt % 2)
                # per-kb trim everywhere; the AV pair reads [pq0, 512) so the
                # odd member's [pq0, q0_own) masked span is zero-filled by a
                # cheap DVE memset instead of widened exp work
                q0 = dlt * P if (diag and dlt > 0) else 0
                if j == 0:
                    et = ep16.tile([P, 2, QW], BF16, name="e16_t")
                    ets.append((et, q0))
                elif kb % 2 == 0:
                    ep = ep8.tile([P, 2, 2, QW], F8E4, name="e8_t")
                    ets.append((ep, q0))
                elif diag and dlt % 2 == 1 and q0 > pdlt * P:
                    ep, _ = ets[kb // 2]
                    nc.vector.memset(ep[:, 1, :, pdlt * P:q0], 0.0)
                sps = ps_s.tile([P, 2, QW], F32, name="s_ps")
                for half in (0, 1):
                    rows = slice(half * DK, (half + 1) * DK)
                    nc.tensor.matmul(
                        sps[:, half, q0:QW],
                        KT[rows, hp, kb * P:(kb + 1) * P],
                        QT[rows, hp, j * QW + q0:(j + 1) * QW],
                        start=True,
                        stop=(not diag),
                    )
                if diag:
                    # PE-side additive causal mask (-30 above the diagonal)
                    for half in (0, 1):
                        nc.tensor.matmul(
                            sps[:, half, dlt * P:(dlt + 1) * P],
                            id_t, btri_t, start=False, stop=True,
                        )
                if j == 0:
                    et, _ = ets[kb]
                    nc.scalar.activation(
                        et[:, :, q0:QW], sps[:, :, q0:QW],
                        mybir.ActivationFunctionType.Exp,
                    )
                else:
                    ep, _ = ets[kb // 2]
                    nc.scalar.activation(
                        ep[:, kb % 2, :, q0:QW], sps[:, :, q0:QW],
                        mybir.ActivationFunctionType.Exp,
                    )

            def emit_readout(j, hp, half, avp, last=False):
                qs = slice(j * QW, (j + 1) * QW)
                avs = dpool.tile([DK, QW], F32, name="avs_t")
                den = dpool.tile([1, QW], F32, name="den_t")
                dscale = (1.0 / AVSCALE) if j != 0 else 1.0
                if last:
                    # ACT is idle after the final exp: evacuate PSUM there
                    nc.scalar.copy(out=avs, in_=avp[0:DK, :])
                    nc.scalar.mul(den, avp[DK:DK + 1, :], dscale)
                else:
                    nc.vector.tensor_copy(out=avs, in_=avp[0:DK, :])
                    # fold the fp8 x16 AVT scale into the denominator so the
                    # normalize is a plain tensor_mul
                    nc.vector.tensor_scalar_mul(den, avp[DK:DK + 1, :], dscale)
                rec = dpool.tile([1, QW], F32, name="rec_t")
                nc.vector.reciprocal_approx_fast(out=rec, in_=den)
                bc = dpool.tile([DK, QW], F32, name="bc_t")
                nc.gpsimd.partition_broadcast(bc, rec)
                dst = (AVT16[half * DK:(half + 1) * DK, hp, :] if j == 0
                       else AVT8[half * DK:(half + 1) * DK, hp, qs])
                nc.vector.tensor_mul(dst, avs, bc)

            def av_generator(j, hp, ets, on_done):
                for half in (0, 1):
                    h = 2 * hp + half
                    avp = ps_v.tile([P, QW], F32, name="av_ps")
                    n = len(ets)
                    if j == 0:
                        for kb, (et, q0) in enumerate(ets):
                            nc.tensor.matmul(
                                avp[0:DK + 1, q0:QW],
                                Vp16[:, kb, h, :],
                                et[:, half, q0:QW],
                                start=(kb == 0),
                                stop=(kb == n - 1),
                            )
                            yield
                    else:
                        for i, (ep, q0) in enumerate(ets):
                            nc.tensor.matmul(
                                avp[0:DK + 2, q0:QW],
                                Vp8[:, i, :, h, 0:DK + 2],
                                ep[:, :, half, q0:QW],
                                start=(i == 0),
                                stop=(i == n - 1),
                                perf_mode=DR,
                            )
                            yield
                    emit_readout(j, hp, half, avp,
                                 last=((j, hp) == (2, 1) and causal))
                on_done()

            def final_unit(qn, m):
                ps = ps_mm.tile([P, QW], F32, name="mm_ps")
                ot = opool.tile([P, QW], BF16, name="o_t")
                if qn == 0:
                    for cs in range(2):
                        nc.tensor.matmul(
                            ps,
                            wo16_r[:, cs, m * P:(m + 1) * P],
                            AVT16[:, cs, :],
                            start=(cs == 0),
                            stop=(cs == 1),
                        )
                    nc.vector.tensor_copy(out=ot, in_=ps)
                    nc.sync.dma_start(
                        out=outT[m * P:(m + 1) * P, 0:QW], in_=ot)
                elif qn == 2:
                    # last-emitted group: drain in parallel — even tiles on
                    # the then-idle ACT engine + its queue, odd tiles on DVE
                    # + sync, halving the serial tail chain
                    nc.tensor.matmul(
                        ps,
                        wo_r[:, :, m * P:(m + 1) * P],
                        AVT8[:, :, qn * QW:(qn + 1) * QW],
                        start=True, stop=True, perf_mode=DR,
                    )
                    if m % 2 == 0:
                        nc.scalar.mul(ot, ps, OSCALE8)
                        nc.scalar.dma_start(
                            out=outT[m * P:(m + 1) * P, qn * QW:(qn + 1) * QW],
                            in_=ot)
                    else:
                        nc.vector.tensor_scalar_mul(ot, ps, OSCALE8)
                        nc.sync.dma_start(
                            out=outT[m * P:(m + 1) * P, qn * QW:(qn + 1) * QW],
                            in_=ot)
                else:
                    nc.tensor.matmul(
                        ps,
                        wo_r[:, :, m * P:(m + 1) * P],
                        AVT8[:, :, qn * QW:(qn + 1) * QW],
                        start=True, stop=True, perf_mode=DR,
                    )
                    nc.vector.tensor_scalar_mul(ot, ps, OSCALE8)
                    nc.sync.dma_start(
                        out=outT[m * P:(m + 1) * P, qn * QW:(qn + 1) * QW],
                        in_=ot)

            # ---------- schedule ----------
            filler = []
            released = set()

            def fill(n=1):
                done = 0
                i = 0
                while done < n and i < len(filler):
                    tag, fn = filler[i]
                    if tag and tag not in released:
                        i += 1
                        continue
                    filler.pop(i)
                    fn()
                    done += 1

            def mk_chunk(which, th, cs, t2):
                return lambda: proj_qk_chunk(which, th, cs, t2)

            def mk_v(th, t8):
                return lambda: proj_v_tile(th, t8)

            def mk_f(qn, m):
                return lambda: final_unit(qn, m)

            # warm up the PE during the initial DMA wait: the clock needs
            # ~3us of continuous execution to ramp 0.65 -> 2.4 GHz, so burn
            # dummy matmuls on a memset tile until the entry inputs land
            warm = const.tile([P, DK], BF16, name="warm_t")
            nc.vector.memset(warm, 0.125)
            for _ in range(64):
                wps = ps_mm.tile([P, QW], F32, name="mm_ps")
                nc.tensor.matmul(wps[0:DK, 0:DK], warm, warm,
                                 start=True, stop=True)

            # entry: only what S(0,0) reads (q/k columns [0:512], head
            # pair 0, bf16 path); everything else is filler in first-use
            # order, emitted AFTER each score step so scores lead the queue
            proj_qk_chunk("q", 0, 0, 0)
            proj_qk_chunk("k", 0, 0, 0)
            filler.append(("", mk_chunk("q", 0, 1, 0)))   # S(0,1)
            filler.append(("", mk_chunk("k", 0, 1, 0)))
            for t8 in range(8):
                filler.append(("", mk_v(0, t8)))          # A(0,*)
            filler.append(("", mk_chunk("q", 0, 0, 1)))   # S(1,0)
            filler.append(("", mk_chunk("k", 0, 0, 1)))
            filler.append(("", mk_chunk("q", 0, 1, 1)))   # S(1,1)
            filler.append(("", mk_chunk("k", 0, 1, 1)))
            for t2 in (0, 1):
                filler.append(("", mk_chunk("q", 1, 0, t2)))
            for t2 in (0, 1):
                filler.append(("", mk_chunk("k", 1, 0, t2)))
            for t2 in (0, 1):
                filler.append(("", mk_chunk("q", 1, 1, t2)))
            for t2 in (0, 1):
                filler.append(("", mk_chunk("k", 1, 1, t2)))
            for t8 in range(8):
                filler.append(("", mk_v(1, t8)))
            for qn in range(NQ):
                for m in range(ND):
                    filler.append((f"F{qn}", mk_f(qn, m)))

            groups = [(0, 0), (0, 1), (1, 0), (1, 1), (3, 0), (3, 1), (2, 0), (2, 1)]
            prev_gen = None
            prev_n = 0
            for (j, hp) in groups:
                nkb = 4 * j + 4 if causal else NTOK
                ets = []
                done_av = 0
                for kb in range(nkb):
                    score_step(j, hp, kb, ets)
                    if prev_gen is not None:
                        target = ((kb + 1) * prev_n) // nkb
                        while done_av < target:
                            next(prev_gen, None)
                            done_av += 1
                    fill(2)
                if prev_gen is not None:
                    while done_av < prev_n:
                        next(prev_gen, None)
                        done_av += 1
                    next(prev_gen, None)  # trailing readout + on_done

                def mk_done(jj):
                    return lambda: released.add(f"F{jj}")

                prev_gen = av_generator(j, hp, ets,
                                        mk_done(j) if hp == 1 else (lambda: None))
                prev_n = 2 * len(ets)
            for _ in range(prev_n):
                next(prev_gen, None)
                fill(1)
            next(prev_gen, None)
            fill(len(filler) + 1)

    nc.compile()
    return nc


_NC_CACHE = {}


def _get_nc(causal: bool):
    if causal not in _NC_CACHE:
        _NC_CACHE[causal] = build_attention_nc(causal)
    return _NC_CACHE[causal]


def _relay_w(WT):
    # WT is (d_in, c): -> [128, (d, c)] so each partition row is contiguous
    return np.ascontiguousarray(
        WT.reshape(ND, P, -1).transpose(1, 0, 2).reshape(P, -1)
    )


def _relay_x(xT):
    # xT is (D, S): -> [128, (t, th, two, s)] matching x-tile DMA slices
    a = xT.reshape(NDP, 2, P, 2, S // 2)        # t, two, p, th, s
    return np.ascontiguousarray(
        a.transpose(2, 0, 3, 1, 4).reshape(P, ND * S)
    )


def build_in_maps(query, key, value, Wq, bq, Wk, bk, Wv, Wo, causal):
    f8 = ml_dtypes.float8_e4m3fn
    bf = ml_dtypes.bfloat16
    kk = np.arange(P)[:, None]
    qq = np.arange(P)[None, :]
    tri = np.where(kk > qq, np.float32(MB), np.float32(0.0))
    if not causal:
        tri = np.zeros((P, P), np.float32)
    bext_np = np.concatenate([np.full((P, P), MB, np.float32), tri], axis=1)
    ident_np = np.eye(P, dtype=np.float32)

    xT = {n: [np.ascontiguousarray(a[b].T) for b in range(2)]
          for n, a in (("q", query), ("k", key), ("v", value))}
    x8 = {n: [_relay_x(xT[n][b]).astype(f8) for b in range(2)] for n in xT}
    x16 = {n: [_relay_w(xT[n][b][:, 0:QW]).astype(bf) for b in range(2)] for n in xT}

    WqT = np.ascontiguousarray(Wq.T)
    WkT = np.ascontiguousarray(Wk.T)
    WvT = np.ascontiguousarray(Wv.T)
    WoT = np.ascontiguousarray(Wo.T)

    in_maps = []
    for core in range(8):
        b, g = divmod(core, 4)
        cols = slice(g * C, (g + 1) * C)
        wo_core = WoT[cols, :]  # (256, 1024)

        def relay_wo(w):
            return np.ascontiguousarray(
                w.reshape(2, P, D).transpose(1, 0, 2).reshape(P, 2 * D)
            )
        in_maps.append({
            "xq": x8["q"][b], "xk": x8["k"][b], "xv": x8["v"][b],
            "xq16": x16["q"][b], "xk16": x16["k"][b], "xv16": x16["v"][b],
            "wq": _relay_w(WqT[:, cols] * WSCALE).astype(f8),
            "wk": _relay_w(WkT[:, cols] * WSCALE).astype(f8),
            "wv": _relay_w(WvT[:, cols] * WSCALE).astype(f8),
            "wo": relay_wo(wo_core * WSCALE).astype(f8),
            "wq16": _relay_w(WqT[:, cols]).astype(bf),
            "wk16": _relay_w(WkT[:, cols]).astype(bf),
            "wv16": _relay_w(WvT[:, cols]).astype(bf),
            "wo16": relay_wo(wo_core).astype(bf),
            "bq": np.ascontiguousarray((bq[cols] / 8.0).reshape(2, P).T),
            "bk": np.ascontiguousarray(bk[cols].reshape(2, P).T),
            "ident": ident_np.astype(bf),
            "btri": tri.astype(bf),
            "bext": bext_np.astype(bf),
        })
    return in_maps


def kernel(query, key, value, mask, Wq, bq, Wk, bk, Wv, bv, Wo, bo):
    query = np.asarray(query, np.float32)
    key = np.asarray(key, np.float32)
    value = np.asarray(value, np.float32)
    Wq = np.asarray(Wq, np.float32)
    Wk = np.asarray(Wk, np.float32)
    Wv = np.asarray(Wv, np.float32)
    Wo = np.asarray(Wo, np.float32)
    bq = np.asarray(bq, np.float32)
    bk = np.asarray(bk, np.float32)
    bv = np.asarray(bv, np.float32)
    bo = np.asarray(bo, np.float32)
    mask_np = np.asarray(mask)

    causal = bool(mask_np.any())
    if causal:
        idx = np.arange(S)
        expect = idx[None, :] > idx[:, None]
        if not np.array_equal(mask_np.reshape(S, S), expect):
            raise ValueError("kernel only supports the causal (or empty) mask")
    nc = _get_nc(causal)

    in_maps = build_in_maps(query, key, value, Wq, bq, Wk, bk, Wv, Wo, causal)

    res = run_bass_kernel_spmd(nc, in_maps, core_ids=list(range(8)))

    # softmax rows sum to 1, so the V bias contributes bv @ Wo.T to every row.
    bo_eff = bo + bv @ Wo.T
    out = np.empty((2, S, D), np.float32)
    for b in range(2):
        acc = res.results[b * 4]["outT"].astype(np.float32)
        for g in range(1, 4):
            acc += res.results[b * 4 + g]["outT"].astype(np.float32)
        out[b] = acc.T.astype(np.float32) + bo_eff
    return out
